# revision 3
# baseline (speedup 1.0000x reference)
"""ConvCheb (K=3) Trainium2 kernel v3: hybrid vertex x batch sharding.

8 cores = 2 subgroups x 4 cores. Subgroup g handles batches [4g, 4g+4);
within it, core with range index r4 = c%4 owns dest vertices
[r4*R, (r4+1)*R), R = V/4.

Math (W commutes with the vertex-space operator L):
    out = x0 Wa + y1 Wb + (L v)        with y1 = L x0, v = y1 Wc,
    Wa = W0 - W2, Wb = W1, Wc = 2 W2, bias added at the end.

Per core:
  P1: y1^T blocks via scatter-matmul (host-pregathered x0 slots, 4 batches
      wide = 256 cols); psum gives y1t [128(bi,f), 64 rows] per pair ->
      kept resident in SBUF. v = y1 @ Wc via blockdiag(Wc,Wc) matmuls ->
      v_my quarters in DRAM.
  EX: 4 quarter AllGathers (groups [[0..3],[4..7]]) build w = full-V v
      (row order: quarter-major, then range, then local row).
  P2: out rows = x0 Wa + y1 Wb (dense, from host x0t + resident y1t)
      + scatter-matmul over dma_gathered v rows (512B elems, int16 idx
      via lo/hi half split of w) + bias; f32 out.

Uniform SPMD schedule: per-block chunk counts K1/K2lo/K2hi are the max
over the 4 range plans; shorter cores pad with val=0 slots.
"""
import sys
for _p in ("/opt/trn_rl_repo",):
    if _p not in sys.path:
        sys.path.append(_p)
import os
import numpy as np
import ml_dtypes
import concourse.bass as bass
import concourse.bacc as bacc
import concourse.mybir as mybir
import concourse.tile as tile

dt = mybir.dt
F32 = dt.float32
BF16 = dt.bfloat16
NPBF16 = ml_dtypes.bfloat16
FP8 = dt.float8e4

V = 49152
B = 8
F = 64
R = V // 4            # dest rows per core
NB = R // 64          # 192 dest blocks of 64 rows
VH = V // 2           # lo/hi split of the gathered w tensor
Q4 = R // 4           # quarter rows per core shard
CP = 16               # pass-1 chunks per streamed piece
CP2 = 16              # pass-2 chunks per piece
NQ = 4                # SWDGE queues for gathers
GG = 8                # blocks staged per v/out DMA
PREP = os.environ.get("KPREP", "0") == "1"  # prepare_only descriptor pre-gen
KPHASE = os.environ.get("KPHASE", "full")  # full | dense | dumpw


def _w_row(v):
    """Global vertex id -> row in the AllGathered w tensor."""
    k4 = v // R
    lv = v % R
    q = lv // Q4
    return q * R + k4 * Q4 + (lv % Q4)


def slots_to_chunk_layout(arr):
    """[nslots(, d)] -> [128, nchunks(, d)]: slot j -> [j%128, j//128]."""
    n = arr.shape[0] // 128
    a = arr.reshape(n, 128, *arr.shape[1:])
    return np.ascontiguousarray(np.moveaxis(a, 1, 0))


def wrap_idx16(idx):
    """dma_gather idx layout [128, n/16] int16: idx j at [j%16, j//16],
    replicated across the 8 groups of 16 partitions."""
    n = len(idx)
    assert n % 128 == 0
    w = np.zeros((16, n // 16), np.int16)
    for p in range(16):
        w[p, :] = idx[p::16]
    return np.ascontiguousarray(np.tile(w, (8, 1)))


def make_s(radj, val):
    """[C,128] radj/val -> dense scatter matrices [128, C, 64] bf16."""
    C = radj.shape[0]
    s = np.zeros((C, 128, 64), np.float32)
    ci = np.repeat(np.arange(C), 128)
    si = np.tile(np.arange(128), C)
    s[ci, si, radj.reshape(-1)] = val.reshape(-1)
    return np.ascontiguousarray(np.moveaxis(s, 1, 0)).astype(NPBF16)


def _pad_to(arr, n, fill):
    pad = n - len(arr)
    if pad <= 0:
        return arr
    return np.concatenate([arr, np.full(pad, fill, arr.dtype)])


def host_prepare(rows, cols, vals):
    """Graph-dependent tables, shared by both subgroups (4 range plans)."""
    blk_all = rows // 64  # global dest block; r4 = blk_all // NB? no: rows//R
    per = []
    for r4 in range(4):
        m = (rows // R) == r4
        r_l = (rows[m] - r4 * R).astype(np.int64)
        c_g = cols[m].astype(np.int64)
        v = vals[m].astype(np.float32)
        per.append((r_l, c_g, v))

    # ---- pass 1: sort by dest block ----
    p1 = []
    cnt1 = np.zeros((4, NB), np.int64)
    for r4, (r_l, c_g, v) in enumerate(per):
        o = np.argsort(r_l // 64, kind="stable")
        r_s, c_s, v_s = r_l[o], c_g[o], v[o]
        b = r_s // 64
        lo = np.searchsorted(b, np.arange(NB), "left")
        hi = np.searchsorted(b, np.arange(NB), "right")
        cnt1[r4] = hi - lo
        p1.append((r_s, c_s, v_s, lo, hi))
    K1 = np.maximum(1, np.ceil(cnt1.max(axis=0) / 128).astype(np.int64))
    C1 = int(K1.sum())

    # ---- pass 2: sort by (dest block, w-half) ----
    p2 = []
    cnt2 = np.zeros((4, NB, 2), np.int64)
    for r4, (r_l, c_g, v) in enumerate(per):
        wr = _w_row(c_g)
        half = (wr >= VH).astype(np.int64)
        key = (r_l // 64) * 2 + half
        o = np.argsort(key, kind="stable")
        r_s, w_s, v_s, k_s = r_l[o], wr[o], v[o], key[o]
        lo = np.searchsorted(k_s, np.arange(2 * NB), "left")
        hi = np.searchsorted(k_s, np.arange(2 * NB), "right")
        cnt2[r4] = (hi - lo).reshape(NB, 2)
        p2.append((r_s, w_s, v_s, lo, hi))
    K2 = np.ceil(cnt2.max(axis=0) / 128).astype(np.int64)  # [NB, 2]
    K2 = np.maximum(K2, 1)

    # global chunk template (block-major), then per-piece lo-first permute
    entries = []  # (block, half)
    for b in range(NB):
        entries += [(b, 0)] * int(K2[b, 0]) + [(b, 1)] * int(K2[b, 1])
    C2 = len(entries)
    perm = []       # new position -> original entry index
    piece_tab = []  # (nlo, nhi) per piece
    for p0 in range(0, C2, CP2):
        seg = list(range(p0, min(p0 + CP2, C2)))
        lo_e = [i for i in seg if entries[i][1] == 0]
        hi_e = [i for i in seg if entries[i][1] == 1]
        piece_tab.append((len(lo_e), len(hi_e)))
        perm.append(lo_e + hi_e)
    perm = [i for pc in perm for i in pc]
    newpos = np.empty(C2, np.int64)
    newpos[np.array(perm)] = np.arange(C2)
    block_chunks = [[] for _ in range(NB)]
    for orig_i, (b, _h) in enumerate(entries):
        block_chunks[b].append(int(newpos[orig_i]))
    for b in range(NB):
        block_chunks[b].sort()

    # ---- per-range slot arrays in final chunk orders ----
    cols1 = []   # per r4: [C1*128] global x0 row per slot (pass-1 host gather)
    s1 = []
    s2 = []
    idxw = []
    for r4 in range(4):
        r_s, c_s, v_s, lo, hi = p1[r4]
        ca = np.zeros(C1 * 128, np.int64)
        ra = np.zeros(C1 * 128, np.int64)
        va = np.zeros(C1 * 128, np.float32)
        off = 0
        for b in range(NB):
            n = int(K1[b]) * 128
            sl = slice(lo[b], hi[b])
            ca[off:off + n] = _pad_to(c_s[sl], n, 0)
            ra[off:off + n] = _pad_to(r_s[sl] - b * 64, n, 0)
            va[off:off + n] = _pad_to(v_s[sl], n, 0.0)
            off += n
        cols1.append(ca)
        s1.append(make_s(ra.reshape(C1, 128), va.reshape(C1, 128)))

        r_s, w_s, v_s, lo, hi = p2[r4]
        ia = np.zeros((C2, 128), np.int64)
        ra = np.zeros((C2, 128), np.int64)
        va = np.zeros((C2, 128), np.float32)
        for b in range(NB):
            for h in range(2):
                g = 2 * b + h
                sl = slice(lo[g], hi[g])
                n = int(K2[b, h]) * 128
                iv = _pad_to(w_s[sl] - h * VH, n, 0)
                rv = _pad_to(r_s[sl] - b * 64, n, 0)
                vv = _pad_to(v_s[sl], n, 0.0)
                base = int(K2[:b].sum()) * 1  # not used; chunks via entries
                # positions of this (b, h) group's chunks in the new order
                ent0 = sum(int(K2[bb, 0] + K2[bb, 1]) for bb in range(b)) + (
                    int(K2[b, 0]) if h == 1 else 0)
                for k in range(int(K2[b, h])):
                    npos = int(newpos[ent0 + k])
                    ia[npos] = iv[k * 128:(k + 1) * 128]
                    ra[npos] = rv[k * 128:(k + 1) * 128]
                    va[npos] = vv[k * 128:(k + 1) * 128]
        s2.append(make_s(ra, va))
        idxw.append(wrap_idx16(ia.reshape(-1).astype(np.int16)))

    return dict(K1=K1, C1=C1, K2=K2, C2=C2, piece_tab=piece_tab,
                block_chunks=block_chunks, cols1=cols1, s1=s1, s2=s2,
                idxw=idxw)


def build_kernel(nc, tabs):
    C1, C2 = tabs["C1"], tabs["C2"]
    K1, K2 = tabs["K1"], tabs["K2"]
    piece_tab = tabs["piece_tab"]
    block_chunks = tabs["block_chunks"]

    g1_d = nc.dram_tensor("g1", [128, C1, 256], BF16, kind="ExternalInput")
    s1_d = nc.dram_tensor("s1", [128, C1, 64], BF16, kind="ExternalInput")
    s2_d = nc.dram_tensor("s2", [128, C2, 64], BF16, kind="ExternalInput")
    idx2_d = nc.dram_tensor("idx2", [128, C2 * 8], dt.int16, kind="ExternalInput")
    x0t_d = nc.dram_tensor("x0t", [128, 2, R], BF16, kind="ExternalInput")
    wm_d = nc.dram_tensor("wm", [128, 3, 128], BF16, kind="ExternalInput")
    bias_d = nc.dram_tensor("biasx", [64, 256], F32, kind="ExternalInput")
    out_d = nc.dram_tensor("outx", [R, 256], F32, kind="ExternalOutput")

    # internal DRAM: per-quarter v shards + the gathered w halves
    v_q = [nc.dram_tensor(f"vq{q}", [Q4, 256], FP8) for q in range(4)]
    w_lo_d = nc.dram_tensor("wlo", [VH, 256], FP8)
    w_hi_d = nc.dram_tensor("whi", [VH, 256], FP8)
    if KPHASE == "dumpw":
        wdbg_lo = nc.dram_tensor("wdbglo", [VH, 256], BF16,
                                 kind="ExternalOutput")
        wdbg_hi = nc.dram_tensor("wdbghi", [VH, 256], BF16,
                                 kind="ExternalOutput")

    with tile.TileContext(nc) as tc:
        with (
            tc.tile_pool(name="const", bufs=1) as cpool,
            tc.tile_pool(name="y1t", bufs=1) as ypool,
            tc.tile_pool(name="g1p", bufs=3) as g1pool,
            tc.tile_pool(name="s1p", bufs=3) as s1pool,
            tc.tile_pool(name="g2p", bufs=4) as g2pool,
            tc.tile_pool(name="g2f", bufs=4) as g2fpool,
            tc.tile_pool(name="s2p", bufs=4) as s2pool,
            tc.tile_pool(name="idxp", bufs=4) as idxpool,
            tc.tile_pool(name="vstg", bufs=2) as vstgpool,
            tc.tile_pool(name="x0s", bufs=3) as x0spool,
            tc.tile_pool(name="ostg", bufs=3) as ostgpool,
            tc.tile_pool(name="psA", bufs=2, space="PSUM") as psA,
            tc.tile_pool(name="psV", bufs=2, space="PSUM") as psV,
            tc.tile_pool(name="psO", bufs=3, space="PSUM") as psO,
        ):
            wm_t = cpool.tile([128, 3, 128], BF16)
            nc.scalar.dma_start(wm_t[:], wm_d.ap())
            bias_t = cpool.tile([64, 256], F32)
            nc.scalar.dma_start(bias_t[:], bias_d.ap())


            # resident y1^T: per (block, pair) a [128, 64] slab
            y1t_t = ypool.tile([128, NB * 128], BF16)

            def ysl(b, pr):
                return y1t_t[:, b * 128 + pr * 64: b * 128 + pr * 64 + 64]

            # ---------- PASS 1 ----------
            g1_tiles, s1_tiles = [], []
            np1 = (C1 + CP - 1) // CP

            def emit_p1(p):
                c0 = p * CP
                w = min(CP, C1 - c0)
                g1_t = g1pool.tile([128, CP, 256], BF16, tag="g1")
                nc.sync.dma_start(g1_t[:, 0:w, :], g1_d.ap()[:, c0:c0 + w, :])
                s1_t = s1pool.tile([128, CP, 64], BF16, tag="s1")
                nc.sync.dma_start(s1_t[:, 0:w, :], s1_d.ap()[:, c0:c0 + w, :])
                g1_tiles.append(g1_t)
                s1_tiles.append(s1_t)

            cglob = 0
            vstage = None
            for b in range(NB):
                pa_t = psA.tile([128, 2, 64], F32, tag="psA", name="pa_t")
                pa = [pa_t[:, 0, :], pa_t[:, 1, :]]
                kb = int(K1[b])
                for j in range(kb):
                    while cglob // CP >= len(g1_tiles):
                        emit_p1(len(g1_tiles))
                    p, cip = divmod(cglob, CP)
                    for pr in (0, 1):
                        nc.tensor.matmul(
                            pa[pr],
                            g1_tiles[p][:, cip, 128 * pr:128 * pr + 128],
                            s1_tiles[p][:, cip, :],
                            start=(j == 0 and pr == 0),
                            stop=(j == kb - 1 and pr == 1))
                    cglob += 1
                for pr in (0, 1):
                    nc.vector.tensor_copy(ysl(b, pr), pa[pr])
                pv = psV.tile([64, 256], F32, tag="psV")
                for pr in (0, 1):
                    nc.tensor.matmul(pv[:, 128 * pr:128 * pr + 128],
                                     ysl(b, pr), wm_t[:, 2, :],
                                     start=(pr == 0), stop=(pr == 1))
                if b % GG == 0:
                    vstage = vstgpool.tile([64, GG, 256], FP8, tag="vst")
                nc.vector.tensor_copy(vstage[:, b % GG, :], pv[:])
                if b % GG == GG - 1:
                    g = b // GG  # 24 stage groups; 6 per quarter
                    q, gq = divmod(g, 6)
                    dst = v_q[q].ap().rearrange("(e a p) c -> e p a c",
                                                e=6, p=64)
                    nc.sync.dma_start(dst[gq], vstage[:])

            # ---------- EXCHANGE: 4 quarter AllGathers ----------
            for q in range(4) if KPHASE != "dense" else []:
                outw = (w_lo_d if q < 2 else w_hi_d)
                qi = q % 2
                nc.gpsimd.collective_compute(
                    "AllGather",
                    mybir.AluOpType.bypass,
                    replica_groups=[[0, 1, 2, 3], [4, 5, 6, 7]],
                    ins=[v_q[q].ap()],
                    outs=[outw.ap().rearrange("(qi r) c -> qi r c", qi=2)[qi]],
                )

            if KPHASE == "dumpw":
                for hf, (srcd, dstd) in enumerate(
                        [(w_lo_d, wdbg_lo), (w_hi_d, wdbg_hi)]):
                    sap = srcd.ap().rearrange("(g a p) c -> g p a c",
                                              g=16, p=128)
                    dap = dstd.ap().rearrange("(g a p) c -> g p a c",
                                              g=16, p=128)
                    for g in range(16):
                        t = vstgpool.tile([128, 12, 256], BF16, tag="wdbg",
                                          name="twd")
                        nc.sync.dma_start(t[:], sap[g])
                        nc.sync.dma_start(dap[g], t[:])

            # ---------- PASS 2 ----------
            g2_tiles, s2_tiles = [], []
            qctr = [0]
            np2 = (C2 + CP - 1) // CP
            w_lo_ap = w_lo_d.ap()
            w_hi_ap = w_hi_d.ap()

            def emit_p2(p):
                c0 = p * CP2
                w = min(CP2, C2 - c0)
                nlo, nhi = piece_tab[p]
                it = idxpool.tile([128, CP2 * 8], dt.int16, tag="idx2")
                nc.scalar.dma_start(it[:, 0:w * 8],
                                    idx2_d.ap()[:, c0 * 8:(c0 + w) * 8])
                s2_t = s2pool.tile([128, CP2, 64], BF16, tag="s2")
                nc.sync.dma_start(s2_t[:, 0:w, :], s2_d.ap()[:, c0:c0 + w, :])
                gt = g2fpool.tile([128, CP2, 256], FP8, tag="g2f")
                if nlo:
                    nc.gpsimd.dma_gather(
                        gt[:, 0:nlo, :], w_lo_ap, it[:, 0:nlo * 8],
                        num_idxs=nlo * 128, num_idxs_reg=nlo * 128,
                        elem_size=256, single_packet=False,
                        queue_num=qctr[0] % NQ)
                    qctr[0] += 1
                if nhi:
                    nc.gpsimd.dma_gather(
                        gt[:, nlo:nlo + nhi, :], w_hi_ap,
                        it[:, nlo * 8:(nlo + nhi) * 8],
                        num_idxs=nhi * 128, num_idxs_reg=nhi * 128,
                        elem_size=256, single_packet=False,
                        queue_num=qctr[0] % NQ)
                    qctr[0] += 1
                gtb = g2pool.tile([128, CP2, 256], BF16, tag="g2")
                nc.vector.tensor_copy(gtb[:, 0:w, :], gt[:, 0:w, :])
                g2_tiles.append(gtb)
                s2_tiles.append(s2_t)

            ostage = None
            x0s_t = None
            for b in range(NB):
                if b % 8 == 0:
                    x0s_t = x0spool.tile([128, 2, 512], BF16, tag="x0s")
                    nc.scalar.dma_start(
                        x0s_t[:],
                        x0t_d.ap()[:, :, 512 * (b // 8):512 * (b // 8) + 512])
                po = psO.tile([64, 256], F32, tag="psO")
                for pr in (0, 1):
                    sl = slice(128 * pr, 128 * pr + 128)
                    nc.tensor.matmul(po[:, sl],
                                     x0s_t[:, pr, 64 * (b % 8):64 * (b % 8) + 64],
                                     wm_t[:, 0, :], start=(pr == 0),
                                     stop=False)
                    nc.tensor.matmul(po[:, sl], ysl(b, pr), wm_t[:, 1, :],
                                     start=False,
                                     stop=(KPHASE == "dense" and pr == 1))
                chl = block_chunks[b] if KPHASE != "dense" else []
                for k, cpos in enumerate(chl):
                    while cpos // CP2 >= len(g2_tiles):
                        emit_p2(len(g2_tiles))
                    p, cip = divmod(cpos, CP2)
                    nc.tensor.matmul(po[:], s2_tiles[p][:, cip, :],
                                     g2_tiles[p][:, cip, :],
                                     start=False, stop=(k == len(chl) - 1))
                if b % GG == 0:
                    ostage = ostgpool.tile([64, GG, 256], F32, tag="ost")
                nc.vector.tensor_tensor(ostage[:, b % GG, :], po[:], bias_t[:],
                                        op=mybir.AluOpType.add)
                if b % GG == GG - 1:
                    g = b // GG
                    dst = out_d.ap().rearrange("(g a p) c -> g p a c",
                                               g=NB // GG, p=64)
                    nc.sync.dma_start(dst[g], ostage[:])


def make_in_map(tabs, inputs_f32, weight, bias, core):
    sub, r4 = divmod(core, 4)
    xs = np.ascontiguousarray(
        inputs_f32[4 * sub:4 * sub + 4].transpose(1, 0, 2).reshape(V, 256)
    ).astype(NPBF16)  # [V, 256] cols = 64*bl + f
    g1 = slots_to_chunk_layout(xs[tabs["cols1"][r4]])
    x0t = np.ascontiguousarray(
        xs[r4 * R:(r4 + 1) * R].T.reshape(2, 128, R).transpose(1, 0, 2))
    wa = (weight[:, 0, :] - weight[:, 2, :]).astype(np.float32)
    wb = weight[:, 1, :].astype(np.float32)
    wc = (2.0 * weight[:, 2, :]).astype(np.float32)
    eye2 = np.eye(2, dtype=np.float32)
    wm = np.stack([np.kron(eye2, wa), np.kron(eye2, wb),
                   np.kron(eye2, wc)], axis=1).astype(NPBF16)
    bias_t = np.tile(bias.astype(np.float32), (64, 4))
    return {
        "g1": np.ascontiguousarray(g1),
        "s1": tabs["s1"][r4],
        "s2": tabs["s2"][r4],
        "idx2": tabs["idxw"][r4],
        "x0t": x0t,
        "wm": np.ascontiguousarray(wm),
        "biasx": np.ascontiguousarray(bias_t),
    }


_KERNEL_CACHE = {}


def _get_compiled(tabs):
    key = "k"
    if key not in _KERNEL_CACHE:
        nc = bacc.Bacc("TRN2", target_bir_lowering=False, debug=False,
                       num_devices=8, num_swdge_queues=NQ)
        build_kernel(nc, tabs)
        nc.compile()
        _KERNEL_CACHE[key] = nc
    return _KERNEL_CACHE[key]


def kernel(inputs, weight, bias, lap_rows, lap_cols, lap_vals):
    from concourse.bass_utils import run_bass_kernel_spmd

    Bi, Vi, Fi = inputs.shape
    assert (Bi, Vi, Fi) == (B, V, F)
    rows = np.asarray(lap_rows).astype(np.int64)
    cols = np.asarray(lap_cols).astype(np.int64)
    vals = np.asarray(lap_vals).astype(np.float32)
    inputs = np.asarray(inputs, dtype=np.float32)
    weight = np.asarray(weight, dtype=np.float32)
    bias = np.asarray(bias, dtype=np.float32)

    tabs = host_prepare(rows, cols, vals)
    nc = _get_compiled(tabs)

    in_maps = [make_in_map(tabs, inputs, weight, bias, c) for c in range(B)]
    res = run_bass_kernel_spmd(nc, in_maps, list(range(B)))
    out = np.empty((B, V, F), np.float32)
    for c in range(B):
        sub, r4 = divmod(c, 4)
        oc = res.results[c]["outx"]  # [R, 256]
        out[4 * sub:4 * sub + 4, r4 * R:(r4 + 1) * R, :] = (
            oc.reshape(R, 4, 64).transpose(1, 0, 2))
    return out
